# revision 1
# baseline (speedup 1.0000x reference)
"""Grouped GEMM (MoE expert-parallel) Trainium2 kernel.

Problem: inp [16384, 4096] f32, weight [8, 4096, 4096] f32 ([e, out_f, in_d]),
tokens pre-grouped by expert, 2048 tokens/expert.
out[e*2048+m, f] = sum_d inp[e*2048+m, d] * weight[e, f, d].

Strategy: expert-parallel, one expert per NeuronCore (8 cores), no
collectives. Host-side layout: each core receives xt = x_e^T [D, M] and
wt = w_e^T [D, F] (both d-major so the contraction dim lands on SBUF
partitions with natural DMAs). The device computes outT = w_e @ x_e^T as
[F, M] with the weight tile stationary ([128d, 128f]) and activations
moving ([128d, 512m]); the host transposes outT back while gathering.

Schedule: split-K zigzag over (kh, mh) blocks - xt resident blocks are
[128, 16, 1024] (8MB) in a 2-slot ring so the next block always prefetches
behind the current one (no pipeline bubble, which cost ~60us in the
full-K variant). kh=0 partials go to a DRAM scratch tensor; kh=1 combines
them during PSUM eviction with a vector add.

Matmuls run as float32r (full-rate fp32 streaming mode: 1 cycle/row vs 4
for strict fp32; ~tf32 mantissa, measured rel err ~1.5e-4).
Per-core: 4096 matmuls ([128k,128f] x [128k,512m]) ~= 980us PE-bound.
"""

import numpy as np

E = 8
M = 2048  # tokens per expert
D = 4096  # in features (contraction)
F = 4096  # out features
P = 128

KO = D // P  # 32 k-subtiles
FO = F // P  # 32 f blocks (stationary tiles per k)
MSEG = 512  # moving free dim per matmul
KH = 2  # split-K passes
KC = KO // KH  # 16 k-subtiles per pass
MB = 1024  # m block resident in SBUF
NMB = M // MB  # 2

_cache = {}


def _build_nc(dtype_tag="f32r"):
    import concourse.bass as bass
    import concourse.mybir as mybir
    import concourse.tile as tile
    from concourse import bacc

    f32 = mybir.dt.float32
    mm_dt = mybir.dt.float32r if dtype_tag == "f32r" else mybir.dt.float32

    nc = bacc.Bacc(None, target_bir_lowering=False, debug=False)

    xt_d = nc.dram_tensor("xt", [D, M], f32, kind="ExternalInput")
    wt_d = nc.dram_tensor("wt", [D, F], f32, kind="ExternalInput")
    ot_d = nc.dram_tensor("ot", [F, M], f32, kind="ExternalOutput")
    pt_d = nc.dram_tensor("ptmp", [F, M], f32)  # kh=0 partial sums

    # d-major -> partition-major views
    xt_r = xt_d[:].rearrange("(ko p) m -> p ko m", p=P)  # [128, 32, 2048]
    wt_r = wt_d[:].rearrange("(ko p) f -> p ko f", p=P)  # [128, 32, 4096]
    ot_r = ot_d[:].rearrange("(fo p) m -> p fo m", p=P)  # [128, 32, 2048]
    pt_r = pt_d[:].rearrange("(fo p) m -> p fo m", p=P)

    n_seg = MB // MSEG  # 2 moving segments per m block

    # zigzag so consecutive blocks differ in exactly one coordinate and the
    # xt ring (bufs=2) always prefetches the next block during the current
    blocks = [(0, 0), (0, 1), (1, 1), (1, 0)]

    with tile.TileContext(nc) as tc:
        with (
            tc.tile_pool(name="xblk", bufs=2) as xblk,
            tc.tile_pool(name="wstream", bufs=3) as wstream,
            tc.tile_pool(name="pin", bufs=3) as pin,
            tc.tile_pool(name="evict", bufs=3) as evict,
            tc.tile_pool(name="psum", bufs=8, space="PSUM") as psum,
        ):
            def load_xt_chunk(tile_, kh_, mh_, kc_):
                nc.sync.dma_start(
                    tile_[:, kc_, :],
                    xt_r[:, kh_ * KC + kc_, mh_ * MB : (mh_ + 1) * MB].bitcast(mm_dt),
                )

            def load_wt(kh_, fo_):
                wt_sb = wstream.tile([P, KC, P], mm_dt, tag="w")
                nc.sync.dma_start(
                    wt_sb[:],
                    wt_r[
                        :, kh_ * KC : (kh_ + 1) * KC, fo_ * P : (fo_ + 1) * P
                    ].bitcast(mm_dt),
                )
                return wt_sb

            next_xt = None
            wt_prefetched = {}
            for bi, (kh, mh) in enumerate(blocks):
                m0 = mh * MB
                fo_range = (
                    list(range(FO)) if bi % 2 == 0 else list(range(FO - 1, -1, -1))
                )
                if bi == 0:
                    # first weight tiles ahead of the bulk xt loads so the
                    # first matmuls aren't queued behind 8MB of activations
                    for fo in fo_range[:2]:
                        wt_prefetched[fo] = load_wt(kh, fo)
                    xt_sb = xblk.tile([P, KC, MB], mm_dt, tag="x")
                    for kc in range(KC):
                        load_xt_chunk(xt_sb, kh, mh, kc)
                else:
                    xt_sb = next_xt
                # next block's resident tile: loads burst on the
                # scalar-engine rings (isolated from the just-in-time wt
                # stream on the sync rings), delayed a few iterations so the
                # current block's own chunks keep ring priority
                if bi + 1 < len(blocks):
                    next_xt = xblk.tile([P, KC, MB], mm_dt, tag="x")
                    for kc in range(KC):
                        load_xt_chunk(next_xt, *blocks[bi + 1], kc)

                for j, fo in enumerate(fo_range):
                    wt_sb = wt_prefetched.pop(fo, None)
                    if wt_sb is None:
                        wt_sb = load_wt(kh, fo)

                    ps = [
                        psum.tile([P, MSEG], f32, tag="acc", name=f"ps_{bi}_{fo}_{s}")
                        for s in range(n_seg)
                    ]
                    for k in range(KC):
                        for s in range(n_seg):
                            nc.tensor.matmul(
                                ps[s],
                                wt_sb[:, k, :],
                                xt_sb[:, k, s * MSEG : (s + 1) * MSEG],
                                start=(k == 0),
                                stop=(k == KC - 1),
                            )

                    ot_sb = evict.tile([P, MB], f32, tag="ev")
                    if kh == 0:
                        for s in range(n_seg):
                            nc.vector.tensor_copy(
                                ot_sb[:, s * MSEG : (s + 1) * MSEG], ps[s]
                            )
                        nc.sync.dma_start(pt_r[:, fo, m0 : m0 + MB], ot_sb[:])
                    else:
                        pin_sb = pin.tile([P, MB], f32, tag="pi")
                        nc.sync.dma_start(pin_sb[:], pt_r[:, fo, m0 : m0 + MB])
                        for s in range(n_seg):
                            nc.vector.tensor_tensor(
                                ot_sb[:, s * MSEG : (s + 1) * MSEG],
                                pin_sb[:, s * MSEG : (s + 1) * MSEG],
                                ps[s],
                                mybir.AluOpType.add,
                            )
                        nc.sync.dma_start(ot_r[:, fo, m0 : m0 + MB], ot_sb[:])

    nc.compile()
    return nc


def _get_nc():
    if "nc" not in _cache:
        _cache["nc"] = _build_nc()
    return _cache["nc"]


def kernel(inp, weight, num_tokens_per_expert):
    from concourse.bass_utils import run_bass_kernel_spmd

    inp = np.asarray(inp)
    weight = np.asarray(weight)
    assert inp.shape == (E * M, D) and weight.shape == (E, F, D)

    nc = _get_nc()
    in_maps = [
        {
            "xt": np.ascontiguousarray(inp[e * M : (e + 1) * M].T),
            "wt": np.ascontiguousarray(weight[e].T),
        }
        for e in range(E)
    ]
    res = run_bass_kernel_spmd(nc, in_maps, list(range(E)))
    out = np.empty((E * M, F), dtype=np.float32)
    for e in range(E):
        out[e * M : (e + 1) * M] = res.results[e]["ot"].T
    return out

